# revision 22
# baseline (speedup 1.0000x reference)
"""Trainium2 Bass kernel for nn_Attention_48653389529249.

Reference (note swapped K/V inputs and softmax over the QUERY axis, dim=1):
    Q = q_input @ Wq.T + bq
    K = v_input @ Wk.T + bk
    V = k_input @ Wv.T + bv
    scores = Q @ K.T / sqrt(256)
    attn = softmax(scores, axis=1)        # over queries
    out = attn @ V
    returns (out, attn)

Shapes: B=4, S=4096, E=256, f32.

Sharding: 8 cores; core c handles batch b = c // 2 and key-range half
h = c % 2 (2048 keys). Scores are computed TRANSPOSED on-chip,
scoresT[k, q], so the softmax (over q) is a free-axis reduction — fully
local to each core.  out = attnT.T @ V is accumulated per core over its
local key range; the host sums the two partial outputs per batch.

The host pre-transposes inputs to [embed, token] layout (the TensorE
contracts over the partition axis, so both matmul operands need the
contraction dim on partitions) and post-transposes the outputs.
Matmul-consumed tensors are typed float32r end-to-end (full-rate PE).
attn is stored/written as bf16 (host casts to f32): ~2e-3 rounding,
halves the dominant write traffic.
"""

import numpy as np
import ml_dtypes

B, S, E = 4, 4096, 256
P = 128
KL = 2048           # local key range per core
NKT = KL // P       # 16 k-tiles of 128
MP = 4              # mega-passes over k-tiles (SBUF capacity for attn tiles)
KT_PER_MP = NKT // MP
NC_CORES = 8
SCALE = 1.0 / 16.0  # 1/sqrt(E)


def _build_program():
    import concourse.bass as bass
    import concourse.tile as tile
    from concourse import bacc, mybir

    f32 = mybir.dt.float32
    f32r = mybir.dt.float32r
    bf16 = mybir.dt.bfloat16
    FT = mybir.ActivationFunctionType
    AX = mybir.AxisListType

    nc = bacc.Bacc("TRN2", target_bir_lowering=False, debug=False,
                   enable_partition_id=False)

    # ---- DRAM parameters (per-core shard, host-prepared layouts) ----
    qT_d = nc.declare_dram_parameter("qT", [E, S], f32r, isOutput=False)
    kTs_d = nc.declare_dram_parameter("kT_src", [E, KL], f32r, isOutput=False)
    vTs_d = nc.declare_dram_parameter("vT_src", [E, KL], f32r, isOutput=False)
    wAll_d = nc.declare_dram_parameter("wAll", [2, P, 3 * E], f32r,
                                       isOutput=False)
    bqk_d = nc.declare_dram_parameter("bqk", [P, 4], f32, isOutput=False)
    bv_d = nc.declare_dram_parameter("bv", [P, E], f32, isOutput=False)
    attn_d = nc.declare_dram_parameter("attn", [KL, S], bf16, isOutput=True)
    invS_d = nc.declare_dram_parameter("invS", [P, NKT], f32, isOutput=True)
    outT_d = nc.declare_dram_parameter("outT", [E, S], f32, isOutput=True)

    QC = 1024            # input-chunk width (tokens) for streamed projections
    NQC = S // QC        # 4 chunks for q
    NKC = KL // QC       # 2 chunks for k/v

    with tile.TileContext(nc) as tc:
        with (
            tc.tile_pool(name="consts", bufs=1) as consts,
            tc.tile_pool(name="persist", bufs=1) as persist,
            tc.tile_pool(name="inp", bufs=5) as inp,
            tc.tile_pool(name="attnp", bufs=2 * KT_PER_MP + 1) as attnp,
            tc.tile_pool(name="softm", bufs=4) as softm,
            tc.tile_pool(name="sps", bufs=2, space="PSUM") as sps,
            tc.tile_pool(name="ops", bufs=2, space="PSUM") as ops,
        ):
            # ---- constants (packed: 4 DMAs total) ----
            wAll_sb = consts.tile([P, 2, 3, E], f32r, tag="wAll")
            bqk_sb = consts.tile([P, 4], f32, tag="bqk")
            bv_sb = consts.tile([P, E], f32, tag="bv")
            nc.sync.dma_start(wAll_sb[:, 0, :, :], wAll_d[0])
            wq_sb = wAll_sb[:, :, 0, :]
            wk_sb = wAll_sb[:, :, 1, :]
            wv_sb = wAll_sb[:, :, 2, :]
            bq_sb = bqk_sb[:, 0:2]
            bk_sb = bqk_sb[:, 2:4]

            # ---- persistent tensors ----
            QT_sb = persist.tile([P, 2, S], f32r, tag="QT")      # [e, et, q]
            KT_sb = persist.tile([P, 2, KL], f32r, tag="KT")     # [e, et, k]
            V_sb = persist.tile([P, NKT, E], bf16, tag="V")      # [k, kt, e]
            outT_acc = persist.tile([P, 2, S], f32, tag="outT")  # [e, et, q]
            invs_all = persist.tile([P, NKT], f32, tag="invs")

            # ---- phase helpers ----
            # A(kt): scoresT -> exp(bf16, UNNORMALIZED) + row-sums; then
            #   invS = 1/sum; V_sb[kt] *= invS (folds softmax normalization
            #   into the out-matmul); DMA exp tile + invS (host normalizes
            #   the attn output).
            # B(mp, qq): outT[:, qq] += sum_{kt in mp} Vbar.T @ expT.
            # Emission interleaves at CHUNK granularity so ScalarE exp work
            # hides behind PE matmul work.
            attn_tiles = {}
            asums = {}

            def a_chunk(kt, qg):
                if qg == 0:
                    at_new = attnp.tile([P, S], bf16, tag="attn")
                    sums_new = softm.tile([P, 4], f32, tag="sums")
                    attn_tiles[kt] = at_new
                    asums[kt] = sums_new
                at, sums = attn_tiles[kt], asums[kt]
                ps = sps.tile([P, QC], f32, tag="sc")
                for s2 in range(2):
                    for et in range(2):
                        nc.tensor.matmul(
                            ps[:, s2 * 512:(s2 + 1) * 512],
                            KT_sb[:, et, kt * P:(kt + 1) * P],
                            QT_sb[:, et, qg * QC + s2 * 512:
                                  qg * QC + (s2 + 1) * 512],
                            start=(et == 0), stop=(et == 1),
                        )
                if qg < 2:
                    nc.scalar.activation(
                        at[:, qg * QC:(qg + 1) * QC], ps[:],
                        FT.Exp, bias=0.0, scale=1.0,
                        accum_out=sums[:, qg:qg + 1],
                    )
                else:
                    nc.scalar.activation(
                        at[:, qg * QC:(qg + 1) * QC], ps[:],
                        FT.Exp, bias=0.0, scale=1.0,
                    )
                    nc.vector.reduce_sum(
                        out=sums[:, qg:qg + 1],
                        in_=at[:, qg * QC:(qg + 1) * QC], axis=AX.X)

            def a_finish(kt):
                at, sums = attn_tiles[kt], asums[kt]
                inv = invs_all[:, kt:kt + 1]
                nc.vector.reduce_sum(out=inv, in_=sums[:], axis=AX.X)
                nc.vector.reciprocal(inv, inv)
                nc.vector.tensor_scalar_mul(V_sb[:, kt, :], V_sb[:, kt, :],
                                            inv)
                nc.gpsimd.dma_start(attn_d[kt * P:(kt + 1) * P, :], at[:])

            def b_half(mp, qq, half):
                qs = slice(qq * 512, (qq + 1) * 512)
                if half == 0:
                    po_new = ops.tile([P, 2, 512], f32, tag="po")
                    b_half.po = po_new
                po = b_half.po
                for i in (2 * half, 2 * half + 1):
                    kt = mp * KT_PER_MP + i
                    for et in range(2):
                        nc.tensor.matmul(
                            po[:, et, :],
                            V_sb[:, kt, et * P:(et + 1) * P],
                            attn_tiles[kt][:, qs],
                            start=(i == 0), stop=(i == KT_PER_MP - 1),
                        )
                if half == 1:
                    dst = outT_acc[:, :, qs]
                    if mp == 0:
                        nc.vector.tensor_copy(dst, po[:])
                    else:
                        nc.vector.tensor_add(dst, dst, po[:])
                    if mp == MP - 1:
                        for et in range(2):
                            nc.sync.dma_start(
                                outT_d[et * P:(et + 1) * P, qs],
                                outT_acc[:, et, qs])

            # ---- projections (just-in-time, DMA-aligned) ----
            qch, kch, vch = {}, {}, {}

            def load_chunk(dst, src_d, cidx, engine):
                ch = inp.tile([P, 2, QC], f32r, tag="inp")
                dst[cidx] = ch
                for dc in range(2):
                    engine.dma_start(
                        ch[:, dc, :], src_d[dc * P:(dc + 1) * P,
                                            cidx * QC:(cidx + 1) * QC])

            def proj_q(qc):
                ch = qch.pop(qc)
                for et in range(2):
                    ps = ops.tile([P, QC], f32, tag="po")
                    for s2 in range(2):
                        for dc in range(2):
                            nc.tensor.matmul(
                                ps[:, s2 * 512:(s2 + 1) * 512],
                                wq_sb[:, dc, et * P:(et + 1) * P],
                                ch[:, dc, s2 * 512:(s2 + 1) * 512],
                                start=(dc == 0), stop=(dc == 1),
                            )
                    nc.vector.tensor_scalar_add(
                        QT_sb[:, et, qc * QC:(qc + 1) * QC], ps[:],
                        bq_sb[:, 0:1] if et == 0 else bq_sb[:, 1:2])

            def proj_k(kc):
                ch = kch.pop(kc)
                for et in range(2):
                    ps = ops.tile([P, QC], f32, tag="po")
                    for s2 in range(2):
                        for dc in range(2):
                            nc.tensor.matmul(
                                ps[:, s2 * 512:(s2 + 1) * 512],
                                wk_sb[:, dc, et * P:(et + 1) * P],
                                ch[:, dc, s2 * 512:(s2 + 1) * 512],
                                start=(dc == 0), stop=(dc == 1),
                            )
                    nc.vector.tensor_scalar_add(
                        KT_sb[:, et, kc * QC:(kc + 1) * QC], ps[:],
                        bk_sb[:, 0:1] if et == 0 else bk_sb[:, 1:2])

            def v_proj_unit(u):
                kc, k2 = u // 4, u % 4
                ch = vch[kc]
                ps = ops.tile([P, 2, E], f32, tag="po")
                for kk in range(2):
                    for dc in range(2):
                        nc.tensor.matmul(
                            ps[:, kk, :],
                            ch[:, dc, (k2 * 2 + kk) * P:
                               (k2 * 2 + kk + 1) * P],
                            wv_sb[:, dc, :],
                            start=(dc == 0), stop=(dc == 1),
                        )
                for kk in range(2):
                    kt = kc * (QC // P) + k2 * 2 + kk
                    nc.vector.tensor_add(V_sb[:, kt, :], ps[:, kk, :],
                                         bv_sb[:])

            # ---- main: software-pipelined, chunk-granular ----
            # A0 (kt 0-3), qg-major so scores start after the FIRST QT/KT
            # chunks land; projections run just-in-time between passes.
            qc0 = inp.tile([P, 2, QC], f32r, tag="inp")
            qch[0] = qc0
            nc.sync.dma_start(qc0[:, 0, 0:512], qT_d[0:P, 0:512])
            nc.sync.dma_start(wAll_sb[:, 1, :, :], wAll_d[1])
            nc.sync.dma_start(qc0[:, 1, 0:512], qT_d[P:2 * P, 0:512])
            nc.sync.dma_start(qc0[:, 0, 512:QC], qT_d[0:P, 512:QC])
            nc.sync.dma_start(qc0[:, 1, 512:QC], qT_d[P:2 * P, 512:QC])
            nc.sync.dma_start(bqk_sb[:], bqk_d[:])
            nc.sync.dma_start(bv_sb[:], bv_d[:])
            kc0 = inp.tile([P, 2, QC], f32r, tag="inp")
            kch[0] = kc0
            for dcx in range(2):
                nc.sync.dma_start(kc0[:, dcx, 0:512],
                                  kTs_d[dcx * P:(dcx + 1) * P, 0:512])
                nc.sync.dma_start(kc0[:, dcx, 512:QC],
                                  kTs_d[dcx * P:(dcx + 1) * P, 512:QC])
            load_chunk(qch, qT_d, 1, nc.sync)
            load_chunk(vch, vTs_d, 0, nc.sync)
            load_chunk(vch, vTs_d, 1, nc.sync)
            proj_q(0)
            proj_k(0)
            vu = 0
            for qg in range(4):
                if qg < 3:
                    if qg + 2 < NQC:
                        load_chunk(qch, qT_d, qg + 2, nc.sync)
                    proj_q(qg + 1)
                for kt in range(KT_PER_MP):
                    a_chunk(kt, qg)
                    if kt == 1 and vu < 8:
                        v_proj_unit(vu)
                        vu += 1
                    if kt == 3 and vu < 8:
                        v_proj_unit(vu)
                        vu += 1
            load_chunk(kch, kTs_d, 1, nc.sync)
            proj_k(1)
            for kt in range(KT_PER_MP):
                a_finish(kt)
            # B(mp) interleaved with A(mp+1), one A-chunk per B-half-pass.
            for mp in range(MP):
                nxt_chunks = ([(mp + 1) * KT_PER_MP + i // 4, i % 4]
                              for i in range(16)) if mp + 1 < MP else None
                if mp == MP - 1:
                    nc.sync.dma_start(invS_d[:], invs_all[:])
                for qq in range(8):
                    for half in range(2):
                        b_half(mp, qq, half)
                        if nxt_chunks is not None:
                            kt, qg = next(nxt_chunks)
                            a_chunk(kt, qg)
                            if qg == 3:
                                a_finish(kt)

    if not nc.is_finalized():
        nc.finalize()
    return nc


_PROGRAM = None


def _get_program():
    global _PROGRAM
    if _PROGRAM is None:
        _PROGRAM = _build_program()
    return _PROGRAM


def make_in_maps(q_input, k_input, v_input, Wq, bq, Wk, bk, Wv, bv):
    q_input = np.asarray(q_input, dtype=np.float32)
    k_input = np.asarray(k_input, dtype=np.float32)
    v_input = np.asarray(v_input, dtype=np.float32)
    wqT = np.asarray(Wq, dtype=np.float32).T * SCALE
    wkT = np.asarray(Wk, dtype=np.float32).T
    wvT = np.asarray(Wv, dtype=np.float32).T
    wAll = np.empty((2, P, 3 * E), dtype=np.float32)
    for dc in range(2):
        rows = slice(dc * P, (dc + 1) * P)
        wAll[dc, :, 0:E] = wqT[rows]
        wAll[dc, :, E:2 * E] = wkT[rows]
        wAll[dc, :, 2 * E:3 * E] = wvT[rows]
    bq16 = np.asarray(bq, dtype=np.float32) * SCALE
    bk_f = np.asarray(bk, dtype=np.float32)
    bqk = np.stack([bq16[0:P], bq16[P:2 * P], bk_f[0:P], bk_f[P:2 * P]],
                   axis=1).astype(np.float32)
    bv_r = np.broadcast_to(np.asarray(bv, dtype=np.float32), (P, E)).copy()

    in_maps = []
    for c in range(NC_CORES):
        b, h = c // 2, c % 2
        kr = slice(h * KL, (h + 1) * KL)
        in_maps.append({
            "qT": np.ascontiguousarray(q_input[b].T),
            "kT_src": np.ascontiguousarray(v_input[b, kr].T),  # K from v_input
            "vT_src": np.ascontiguousarray(k_input[b, kr].T),  # V from k_input
            "wAll": wAll, "bqk": bqk, "bv": bv_r,
        })
    return in_maps


def assemble_outputs(results):
    attn = np.empty((B, S, S), dtype=np.float32)
    out = np.zeros((B, S, E), dtype=np.float32)
    for c in range(NC_CORES):
        b, h = c // 2, c % 2
        kr = slice(h * KL, (h + 1) * KL)
        inv = np.asarray(results[c]["invS"]).T.reshape(KL)[:, None]
        attn[b, :, kr] = (np.asarray(results[c]["attn"]).astype(np.float32)
                          * inv).T
        out[b] += np.asarray(results[c]["outT"]).astype(np.float32).T
    return out, attn


def run_sharded(inputs, trace=False, **spmd_kwargs):
    from concourse.bass_utils import run_bass_kernel_spmd
    nc = _get_program()
    in_maps = make_in_maps(**inputs)
    res = run_bass_kernel_spmd(nc, in_maps, core_ids=list(range(NC_CORES)),
                               trace=trace, **spmd_kwargs)
    out, attn = assemble_outputs(res.results)
    return (out, attn), res


def kernel(**inputs):
    (out, attn), _ = run_sharded(inputs, trace=False)
    return out, attn


# revision 23
# speedup vs baseline: 1.0567x; 1.0567x over previous
"""Trainium2 Bass kernel for nn_Attention_48653389529249.

Reference (note swapped K/V inputs and softmax over the QUERY axis, dim=1):
    Q = q_input @ Wq.T + bq
    K = v_input @ Wk.T + bk
    V = k_input @ Wv.T + bv
    scores = Q @ K.T / sqrt(256)
    attn = softmax(scores, axis=1)        # over queries
    out = attn @ V
    returns (out, attn)

Shapes: B=4, S=4096, E=256, f32.

Sharding: 8 cores; core c handles batch b = c // 2 and key-range half
h = c % 2 (2048 keys). Scores are computed TRANSPOSED on-chip,
scoresT[k, q], so the softmax (over q) is a free-axis reduction — fully
local to each core.  out = attnT.T @ V is accumulated per core over its
local key range; the host sums the two partial outputs per batch.

The host pre-transposes inputs to [embed, token] layout (the TensorE
contracts over the partition axis, so both matmul operands need the
contraction dim on partitions) and post-transposes the outputs.
Matmul-consumed tensors are typed float32r end-to-end (full-rate PE).
attn is stored/written as bf16 (host casts to f32): ~2e-3 rounding,
halves the dominant write traffic.
"""

import numpy as np
import ml_dtypes

B, S, E = 4, 4096, 256
P = 128
KL = 2048           # local key range per core
NKT = KL // P       # 16 k-tiles of 128
MP = 4              # mega-passes over k-tiles (SBUF capacity for attn tiles)
KT_PER_MP = NKT // MP
NC_CORES = 8
SCALE = 1.0 / 16.0  # 1/sqrt(E)


def _build_program():
    import concourse.bass as bass
    import concourse.tile as tile
    from concourse import bacc, mybir

    f32 = mybir.dt.float32
    f32r = mybir.dt.float32r
    bf16 = mybir.dt.bfloat16
    FT = mybir.ActivationFunctionType
    AX = mybir.AxisListType

    nc = bacc.Bacc("TRN2", target_bir_lowering=False, debug=False,
                   enable_partition_id=False)

    # ---- DRAM parameters (per-core shard, host-prepared layouts) ----
    qT_d = nc.declare_dram_parameter("qT", [E, S], f32r, isOutput=False)
    kTs_d = nc.declare_dram_parameter("kT_src", [E, KL], f32r, isOutput=False)
    vTs_d = nc.declare_dram_parameter("vT_src", [E, KL], f32r, isOutput=False)
    wAll_d = nc.declare_dram_parameter("wAll", [2, P, 3 * E], f32r,
                                       isOutput=False)
    bqk_d = nc.declare_dram_parameter("bqk", [P, 4], f32, isOutput=False)
    bv_d = nc.declare_dram_parameter("bv", [P, E], f32, isOutput=False)
    attn_d = nc.declare_dram_parameter("attn", [KL, S], bf16, isOutput=True)
    invS_d = nc.declare_dram_parameter("invS", [P, NKT], f32, isOutput=True)
    outT_d = nc.declare_dram_parameter("outT", [E, S], f32, isOutput=True)

    QC = 1024            # input-chunk width (tokens) for streamed projections
    NQC = S // QC        # 4 chunks for q
    NKC = KL // QC       # 2 chunks for k/v

    with tile.TileContext(nc) as tc:
        with (
            tc.tile_pool(name="consts", bufs=1) as consts,
            tc.tile_pool(name="persist", bufs=1) as persist,
            tc.tile_pool(name="inp", bufs=5) as inp,
            tc.tile_pool(name="attnp", bufs=2 * KT_PER_MP + 1) as attnp,
            tc.tile_pool(name="softm", bufs=4) as softm,
            tc.tile_pool(name="sps", bufs=2, space="PSUM") as sps,
            tc.tile_pool(name="ops", bufs=2, space="PSUM") as ops,
        ):
            # ---- constants (packed: 4 DMAs total) ----
            wAll_sb = consts.tile([P, 2, 3, E], f32r, tag="wAll")
            bqk_sb = consts.tile([P, 4], f32, tag="bqk")
            bv_sb = consts.tile([P, E], f32, tag="bv")
            nc.sync.dma_start(wAll_sb[:, 0, :, :], wAll_d[0])
            wq_sb = wAll_sb[:, :, 0, :]
            wk_sb = wAll_sb[:, :, 1, :]
            wv_sb = wAll_sb[:, :, 2, :]
            bq_sb = bqk_sb[:, 0:2]
            bk_sb = bqk_sb[:, 2:4]

            # ---- persistent tensors ----
            QT_sb = persist.tile([P, 2, S], f32r, tag="QT")      # [e, et, q]
            KT_sb = persist.tile([P, 2, KL], f32r, tag="KT")     # [e, et, k]
            V_sb = persist.tile([P, NKT, E], bf16, tag="V")      # [k, kt, e]
            outT_acc = persist.tile([P, 2, S], f32, tag="outT")  # [e, et, q]
            invs_all = persist.tile([P, NKT], f32, tag="invs")

            # ---- phase helpers ----
            # A(kt): scoresT -> exp(bf16, UNNORMALIZED) + row-sums; then
            #   invS = 1/sum; V_sb[kt] *= invS (folds softmax normalization
            #   into the out-matmul); DMA exp tile + invS (host normalizes
            #   the attn output).
            # B(mp, qq): outT[:, qq] += sum_{kt in mp} Vbar.T @ expT.
            # Emission interleaves at CHUNK granularity so ScalarE exp work
            # hides behind PE matmul work.
            attn_tiles = {}
            asums = {}

            def a_chunk(kt, qg):
                if qg == 0:
                    at_new = attnp.tile([P, S], bf16, tag="attn")
                    sums_new = softm.tile([P, 4], f32, tag="sums")
                    attn_tiles[kt] = at_new
                    asums[kt] = sums_new
                at, sums = attn_tiles[kt], asums[kt]
                ps = sps.tile([P, QC], f32, tag="sc")
                for s2 in range(2):
                    for et in range(2):
                        nc.tensor.matmul(
                            ps[:, s2 * 512:(s2 + 1) * 512],
                            KT_sb[:, et, kt * P:(kt + 1) * P],
                            QT_sb[:, et, qg * QC + s2 * 512:
                                  qg * QC + (s2 + 1) * 512],
                            start=(et == 0), stop=(et == 1),
                        )
                nc.scalar.activation(
                    at[:, qg * QC:(qg + 1) * QC], ps[:],
                    FT.Exp, bias=0.0, scale=1.0,
                    accum_out=sums[:, qg:qg + 1],
                )

            def a_finish(kt):
                at, sums = attn_tiles[kt], asums[kt]
                inv = invs_all[:, kt:kt + 1]
                nc.vector.reduce_sum(out=inv, in_=sums[:], axis=AX.X)
                nc.vector.reciprocal(inv, inv)
                nc.vector.tensor_scalar_mul(V_sb[:, kt, :], V_sb[:, kt, :],
                                            inv)
                nc.gpsimd.dma_start(attn_d[kt * P:(kt + 1) * P, :], at[:])

            def b_half(mp, qq, half):
                qs = slice(qq * 512, (qq + 1) * 512)
                if half == 0:
                    po_new = ops.tile([P, 2, 512], f32, tag="po")
                    b_half.po = po_new
                po = b_half.po
                for i in (2 * half, 2 * half + 1):
                    kt = mp * KT_PER_MP + i
                    for et in range(2):
                        nc.tensor.matmul(
                            po[:, et, :],
                            V_sb[:, kt, et * P:(et + 1) * P],
                            attn_tiles[kt][:, qs],
                            start=(i == 0), stop=(i == KT_PER_MP - 1),
                        )
                if half == 1:
                    dst = outT_acc[:, :, qs]
                    if mp == 0:
                        nc.vector.tensor_copy(dst, po[:])
                    else:
                        nc.vector.tensor_add(dst, dst, po[:])
                    if mp == MP - 1:
                        for et in range(2):
                            nc.sync.dma_start(
                                outT_d[et * P:(et + 1) * P, qs],
                                outT_acc[:, et, qs])

            # ---- projections (just-in-time, DMA-aligned) ----
            qch, kch, vch = {}, {}, {}

            def load_chunk(dst, src_d, cidx, engine):
                ch = inp.tile([P, 2, QC], f32r, tag="inp")
                dst[cidx] = ch
                for dc in range(2):
                    engine.dma_start(
                        ch[:, dc, :], src_d[dc * P:(dc + 1) * P,
                                            cidx * QC:(cidx + 1) * QC])

            def proj_q(qc):
                ch = qch.pop(qc)
                for et in range(2):
                    ps = ops.tile([P, QC], f32, tag="po")
                    for s2 in range(2):
                        for dc in range(2):
                            nc.tensor.matmul(
                                ps[:, s2 * 512:(s2 + 1) * 512],
                                wq_sb[:, dc, et * P:(et + 1) * P],
                                ch[:, dc, s2 * 512:(s2 + 1) * 512],
                                start=(dc == 0), stop=(dc == 1),
                            )
                    nc.vector.tensor_scalar_add(
                        QT_sb[:, et, qc * QC:(qc + 1) * QC], ps[:],
                        bq_sb[:, 0:1] if et == 0 else bq_sb[:, 1:2])

            def proj_k(kc):
                ch = kch.pop(kc)
                for et in range(2):
                    ps = ops.tile([P, QC], f32, tag="po")
                    for s2 in range(2):
                        for dc in range(2):
                            nc.tensor.matmul(
                                ps[:, s2 * 512:(s2 + 1) * 512],
                                wk_sb[:, dc, et * P:(et + 1) * P],
                                ch[:, dc, s2 * 512:(s2 + 1) * 512],
                                start=(dc == 0), stop=(dc == 1),
                            )
                    nc.vector.tensor_scalar_add(
                        KT_sb[:, et, kc * QC:(kc + 1) * QC], ps[:],
                        bk_sb[:, 0:1] if et == 0 else bk_sb[:, 1:2])

            def v_proj_unit(u):
                kc, k2 = u // 4, u % 4
                ch = vch[kc]
                ps = ops.tile([P, 2, E], f32, tag="po")
                for kk in range(2):
                    for dc in range(2):
                        nc.tensor.matmul(
                            ps[:, kk, :],
                            ch[:, dc, (k2 * 2 + kk) * P:
                               (k2 * 2 + kk + 1) * P],
                            wv_sb[:, dc, :],
                            start=(dc == 0), stop=(dc == 1),
                        )
                for kk in range(2):
                    kt = kc * (QC // P) + k2 * 2 + kk
                    nc.vector.tensor_add(V_sb[:, kt, :], ps[:, kk, :],
                                         bv_sb[:])

            # ---- main: software-pipelined, chunk-granular ----
            # A0 (kt 0-3), qg-major so scores start after the FIRST QT/KT
            # chunks land; projections run just-in-time between passes.
            qc0 = inp.tile([P, 2, QC], f32r, tag="inp")
            qch[0] = qc0
            nc.sync.dma_start(qc0[:, 0, :], qT_d[0:P, 0:QC])
            nc.sync.dma_start(wAll_sb[:, 1, :, :], wAll_d[1])
            nc.sync.dma_start(qc0[:, 1, :], qT_d[P:2 * P, 0:QC])
            nc.sync.dma_start(bqk_sb[:], bqk_d[:])
            nc.sync.dma_start(bv_sb[:], bv_d[:])
            load_chunk(kch, kTs_d, 0, nc.sync)
            load_chunk(qch, qT_d, 1, nc.sync)
            load_chunk(vch, vTs_d, 0, nc.sync)
            load_chunk(vch, vTs_d, 1, nc.sync)
            proj_q(0)
            proj_k(0)
            vu = 0
            for qg in range(4):
                if qg < 3:
                    if qg + 2 < NQC:
                        load_chunk(qch, qT_d, qg + 2, nc.sync)
                    proj_q(qg + 1)
                for kt in range(KT_PER_MP):
                    a_chunk(kt, qg)
                    if kt == 1 and vu < 8:
                        v_proj_unit(vu)
                        vu += 1
                    if kt == 3 and vu < 8:
                        v_proj_unit(vu)
                        vu += 1
            load_chunk(kch, kTs_d, 1, nc.sync)
            proj_k(1)
            for kt in range(KT_PER_MP):
                a_finish(kt)
            # B(mp) interleaved with A(mp+1), one A-chunk per B-half-pass.
            for mp in range(MP):
                nxt_chunks = ([(mp + 1) * KT_PER_MP + i // 4, i % 4]
                              for i in range(16)) if mp + 1 < MP else None
                for qq in range(8):
                    for half in range(2):
                        b_half(mp, qq, half)
                        if nxt_chunks is not None:
                            kt, qg = next(nxt_chunks)
                            a_chunk(kt, qg)
                            if qg == 3:
                                a_finish(kt)
            nc.sync.dma_start(invS_d[:], invs_all[:])

    if not nc.is_finalized():
        nc.finalize()
    return nc


_PROGRAM = None


def _get_program():
    global _PROGRAM
    if _PROGRAM is None:
        _PROGRAM = _build_program()
    return _PROGRAM


def make_in_maps(q_input, k_input, v_input, Wq, bq, Wk, bk, Wv, bv):
    q_input = np.asarray(q_input, dtype=np.float32)
    k_input = np.asarray(k_input, dtype=np.float32)
    v_input = np.asarray(v_input, dtype=np.float32)
    wqT = np.asarray(Wq, dtype=np.float32).T * SCALE
    wkT = np.asarray(Wk, dtype=np.float32).T
    wvT = np.asarray(Wv, dtype=np.float32).T
    wAll = np.empty((2, P, 3 * E), dtype=np.float32)
    for dc in range(2):
        rows = slice(dc * P, (dc + 1) * P)
        wAll[dc, :, 0:E] = wqT[rows]
        wAll[dc, :, E:2 * E] = wkT[rows]
        wAll[dc, :, 2 * E:3 * E] = wvT[rows]
    bq16 = np.asarray(bq, dtype=np.float32) * SCALE
    bk_f = np.asarray(bk, dtype=np.float32)
    bqk = np.stack([bq16[0:P], bq16[P:2 * P], bk_f[0:P], bk_f[P:2 * P]],
                   axis=1).astype(np.float32)
    bv_r = np.broadcast_to(np.asarray(bv, dtype=np.float32), (P, E)).copy()

    in_maps = []
    for c in range(NC_CORES):
        b, h = c // 2, c % 2
        kr = slice(h * KL, (h + 1) * KL)
        in_maps.append({
            "qT": np.ascontiguousarray(q_input[b].T),
            "kT_src": np.ascontiguousarray(v_input[b, kr].T),  # K from v_input
            "vT_src": np.ascontiguousarray(k_input[b, kr].T),  # V from k_input
            "wAll": wAll, "bqk": bqk, "bv": bv_r,
        })
    return in_maps


def assemble_outputs(results):
    attn = np.empty((B, S, S), dtype=np.float32)
    out = np.zeros((B, S, E), dtype=np.float32)
    for c in range(NC_CORES):
        b, h = c // 2, c % 2
        kr = slice(h * KL, (h + 1) * KL)
        inv = np.asarray(results[c]["invS"]).T.reshape(KL)[:, None]
        attn[b, :, kr] = (np.asarray(results[c]["attn"]).astype(np.float32)
                          * inv).T
        out[b] += np.asarray(results[c]["outT"]).astype(np.float32).T
    return out, attn


def run_sharded(inputs, trace=False, **spmd_kwargs):
    from concourse.bass_utils import run_bass_kernel_spmd
    nc = _get_program()
    in_maps = make_in_maps(**inputs)
    res = run_bass_kernel_spmd(nc, in_maps, core_ids=list(range(NC_CORES)),
                               trace=trace, **spmd_kwargs)
    out, attn = assemble_outputs(res.results)
    return (out, attn), res


def kernel(**inputs):
    (out, attn), _ = run_sharded(inputs, trace=False)
    return out, attn


# revision 24
# speedup vs baseline: 1.0571x; 1.0004x over previous
"""Trainium2 Bass kernel for nn_Attention_48653389529249.

Reference (note swapped K/V inputs and softmax over the QUERY axis, dim=1):
    Q = q_input @ Wq.T + bq
    K = v_input @ Wk.T + bk
    V = k_input @ Wv.T + bv
    scores = Q @ K.T / sqrt(256)
    attn = softmax(scores, axis=1)        # over queries
    out = attn @ V
    returns (out, attn)

Shapes: B=4, S=4096, E=256, f32.

Sharding: 8 cores; core c handles batch b = c // 2 and key-range half
h = c % 2 (2048 keys). Scores are computed TRANSPOSED on-chip,
scoresT[k, q], so the softmax (over q) is a free-axis reduction — fully
local to each core.  out = attnT.T @ V is accumulated per core over its
local key range; the host sums the two partial outputs per batch.

The host pre-transposes inputs to [embed, token] layout (the TensorE
contracts over the partition axis, so both matmul operands need the
contraction dim on partitions) and post-transposes the outputs.
Matmul-consumed tensors are typed float32r end-to-end (full-rate PE).
attn is stored/written as bf16 (host casts to f32): ~2e-3 rounding,
halves the dominant write traffic.
"""

import numpy as np
import ml_dtypes

B, S, E = 4, 4096, 256
P = 128
KL = 2048           # local key range per core
NKT = KL // P       # 16 k-tiles of 128
MP = 4              # mega-passes over k-tiles (SBUF capacity for attn tiles)
KT_PER_MP = NKT // MP
NC_CORES = 8
SCALE = 1.0 / 16.0  # 1/sqrt(E)


def _build_program():
    import concourse.bass as bass
    import concourse.tile as tile
    from concourse import bacc, mybir

    f32 = mybir.dt.float32
    f32r = mybir.dt.float32r
    bf16 = mybir.dt.bfloat16
    FT = mybir.ActivationFunctionType
    AX = mybir.AxisListType

    nc = bacc.Bacc("TRN2", target_bir_lowering=False, debug=False,
                   enable_partition_id=False)

    # ---- DRAM parameters (per-core shard, host-prepared layouts) ----
    qT_d = nc.declare_dram_parameter("qT", [E, S], f32r, isOutput=False)
    kTs_d = nc.declare_dram_parameter("kT_src", [E, KL], f32r, isOutput=False)
    vTs_d = nc.declare_dram_parameter("vT_src", [E, KL], f32r, isOutput=False)
    wAll_d = nc.declare_dram_parameter("wAll", [2, P, 3 * E], f32r,
                                       isOutput=False)
    bqk_d = nc.declare_dram_parameter("bqk", [P, 4], f32, isOutput=False)
    bv_d = nc.declare_dram_parameter("bv", [P, E], f32, isOutput=False)
    attn_d = nc.declare_dram_parameter("attn", [KL, S], bf16, isOutput=True)
    invS_d = nc.declare_dram_parameter("invS", [P, NKT], f32, isOutput=True)
    outT_d = nc.declare_dram_parameter("outT", [E, S], f32, isOutput=True)

    QC = 1024            # input-chunk width (tokens) for streamed projections
    NQC = S // QC        # 4 chunks for q
    NKC = KL // QC       # 2 chunks for k/v

    with tile.TileContext(nc) as tc:
        with (
            tc.tile_pool(name="consts", bufs=1) as consts,
            tc.tile_pool(name="persist", bufs=1) as persist,
            tc.tile_pool(name="inp", bufs=5) as inp,
            tc.tile_pool(name="attnp", bufs=2 * KT_PER_MP + 1) as attnp,
            tc.tile_pool(name="softm", bufs=4) as softm,
            tc.tile_pool(name="sps", bufs=3, space="PSUM") as sps,
            tc.tile_pool(name="ops", bufs=2, space="PSUM") as ops,
        ):
            # ---- constants (packed: 4 DMAs total) ----
            wAll_sb = consts.tile([P, 2, 3, E], f32r, tag="wAll")
            bqk_sb = consts.tile([P, 4], f32, tag="bqk")
            bv_sb = consts.tile([P, E], f32, tag="bv")
            nc.sync.dma_start(wAll_sb[:, 0, :, :], wAll_d[0])
            wq_sb = wAll_sb[:, :, 0, :]
            wk_sb = wAll_sb[:, :, 1, :]
            wv_sb = wAll_sb[:, :, 2, :]
            bq_sb = bqk_sb[:, 0:2]
            bk_sb = bqk_sb[:, 2:4]

            # ---- persistent tensors ----
            QT_sb = persist.tile([P, 2, S], f32r, tag="QT")      # [e, et, q]
            KT_sb = persist.tile([P, 2, KL], f32r, tag="KT")     # [e, et, k]
            V_sb = persist.tile([P, NKT, E], bf16, tag="V")      # [k, kt, e]
            outT_acc = persist.tile([P, 2, S], f32, tag="outT")  # [e, et, q]
            invs_all = persist.tile([P, NKT], f32, tag="invs")

            # ---- phase helpers ----
            # A(kt): scoresT -> exp(bf16, UNNORMALIZED) + row-sums; then
            #   invS = 1/sum; V_sb[kt] *= invS (folds softmax normalization
            #   into the out-matmul); DMA exp tile + invS (host normalizes
            #   the attn output).
            # B(mp, qq): outT[:, qq] += sum_{kt in mp} Vbar.T @ expT.
            # Emission interleaves at CHUNK granularity so ScalarE exp work
            # hides behind PE matmul work.
            attn_tiles = {}
            asums = {}

            def a_chunk(kt, qg):
                if qg == 0:
                    at_new = attnp.tile([P, S], bf16, tag="attn")
                    sums_new = softm.tile([P, 4], f32, tag="sums")
                    attn_tiles[kt] = at_new
                    asums[kt] = sums_new
                at, sums = attn_tiles[kt], asums[kt]
                ps = sps.tile([P, QC], f32, tag="sc")
                for s2 in range(2):
                    for et in range(2):
                        nc.tensor.matmul(
                            ps[:, s2 * 512:(s2 + 1) * 512],
                            KT_sb[:, et, kt * P:(kt + 1) * P],
                            QT_sb[:, et, qg * QC + s2 * 512:
                                  qg * QC + (s2 + 1) * 512],
                            start=(et == 0), stop=(et == 1),
                        )
                nc.scalar.activation(
                    at[:, qg * QC:(qg + 1) * QC], ps[:],
                    FT.Exp, bias=0.0, scale=1.0,
                    accum_out=sums[:, qg:qg + 1],
                )

            def a_finish(kt):
                at, sums = attn_tiles[kt], asums[kt]
                inv = invs_all[:, kt:kt + 1]
                nc.vector.reduce_sum(out=inv, in_=sums[:], axis=AX.X)
                nc.vector.reciprocal(inv, inv)
                nc.vector.tensor_scalar_mul(V_sb[:, kt, :], V_sb[:, kt, :],
                                            inv)
                nc.gpsimd.dma_start(attn_d[kt * P:(kt + 1) * P, :], at[:])

            def b_pass(mp, qq, et):
                qs = slice(qq * 512, (qq + 1) * 512)
                po = ops.tile([P, 512], f32, tag="po")
                for i in range(KT_PER_MP):
                    kt = mp * KT_PER_MP + i
                    nc.tensor.matmul(
                        po[:],
                        V_sb[:, kt, et * P:(et + 1) * P],
                        attn_tiles[kt][:, qs],
                        start=(i == 0), stop=(i == KT_PER_MP - 1),
                    )
                dst = outT_acc[:, et, qs]
                if mp == 0:
                    nc.vector.tensor_copy(dst, po[:])
                else:
                    nc.vector.tensor_add(dst, dst, po[:])
                if mp == MP - 1:
                    nc.sync.dma_start(outT_d[et * P:(et + 1) * P, qs], dst)

            # ---- projections (just-in-time, DMA-aligned) ----
            qch, kch, vch = {}, {}, {}

            def load_chunk(dst, src_d, cidx, engine):
                ch = inp.tile([P, 2, QC], f32r, tag="inp")
                dst[cidx] = ch
                for dc in range(2):
                    engine.dma_start(
                        ch[:, dc, :], src_d[dc * P:(dc + 1) * P,
                                            cidx * QC:(cidx + 1) * QC])

            def proj_q(qc):
                ch = qch.pop(qc)
                for et in range(2):
                    ps = sps.tile([P, QC], f32, tag="sc")
                    for s2 in range(2):
                        for dc in range(2):
                            nc.tensor.matmul(
                                ps[:, s2 * 512:(s2 + 1) * 512],
                                wq_sb[:, dc, et * P:(et + 1) * P],
                                ch[:, dc, s2 * 512:(s2 + 1) * 512],
                                start=(dc == 0), stop=(dc == 1),
                            )
                    nc.vector.tensor_scalar_add(
                        QT_sb[:, et, qc * QC:(qc + 1) * QC], ps[:],
                        bq_sb[:, 0:1] if et == 0 else bq_sb[:, 1:2])

            def proj_k(kc):
                ch = kch.pop(kc)
                for et in range(2):
                    ps = sps.tile([P, QC], f32, tag="sc")
                    for s2 in range(2):
                        for dc in range(2):
                            nc.tensor.matmul(
                                ps[:, s2 * 512:(s2 + 1) * 512],
                                wk_sb[:, dc, et * P:(et + 1) * P],
                                ch[:, dc, s2 * 512:(s2 + 1) * 512],
                                start=(dc == 0), stop=(dc == 1),
                            )
                    nc.vector.tensor_scalar_add(
                        KT_sb[:, et, kc * QC:(kc + 1) * QC], ps[:],
                        bk_sb[:, 0:1] if et == 0 else bk_sb[:, 1:2])

            def v_proj_unit(u):
                kc, k2 = u // 4, u % 4
                ch = vch[kc]
                ps = ops.tile([P, 2, E], f32, tag="po")
                for kk in range(2):
                    for dc in range(2):
                        nc.tensor.matmul(
                            ps[:, kk, :],
                            ch[:, dc, (k2 * 2 + kk) * P:
                               (k2 * 2 + kk + 1) * P],
                            wv_sb[:, dc, :],
                            start=(dc == 0), stop=(dc == 1),
                        )
                for kk in range(2):
                    kt = kc * (QC // P) + k2 * 2 + kk
                    nc.vector.tensor_add(V_sb[:, kt, :], ps[:, kk, :],
                                         bv_sb[:])

            # ---- main: software-pipelined, chunk-granular ----
            # A0 (kt 0-3), qg-major so scores start after the FIRST QT/KT
            # chunks land; projections run just-in-time between passes.
            qc0 = inp.tile([P, 2, QC], f32r, tag="inp")
            qch[0] = qc0
            nc.sync.dma_start(qc0[:, 0, :], qT_d[0:P, 0:QC])
            nc.sync.dma_start(wAll_sb[:, 1, :, :], wAll_d[1])
            nc.sync.dma_start(qc0[:, 1, :], qT_d[P:2 * P, 0:QC])
            nc.sync.dma_start(bqk_sb[:], bqk_d[:])
            nc.sync.dma_start(bv_sb[:], bv_d[:])
            load_chunk(kch, kTs_d, 0, nc.sync)
            load_chunk(qch, qT_d, 1, nc.sync)
            load_chunk(vch, vTs_d, 0, nc.sync)
            load_chunk(vch, vTs_d, 1, nc.sync)
            proj_q(0)
            proj_k(0)
            vu = 0
            for qg in range(4):
                if qg < 3:
                    if qg + 2 < NQC:
                        load_chunk(qch, qT_d, qg + 2, nc.sync)
                    proj_q(qg + 1)
                for kt in range(KT_PER_MP):
                    a_chunk(kt, qg)
                    if kt == 1 and vu < 8:
                        v_proj_unit(vu)
                        vu += 1
                    if kt == 3 and vu < 8:
                        v_proj_unit(vu)
                        vu += 1
            load_chunk(kch, kTs_d, 1, nc.sync)
            proj_k(1)
            for kt in range(KT_PER_MP):
                a_finish(kt)
            # B(mp) interleaved with A(mp+1), one A-chunk per B-half-pass.
            for mp in range(MP):
                nxt_chunks = ([(mp + 1) * KT_PER_MP + i // 4, i % 4]
                              for i in range(16)) if mp + 1 < MP else None
                for qq in range(8):
                    for et in range(2):
                        b_pass(mp, qq, et)
                        if nxt_chunks is not None:
                            kt, qg = next(nxt_chunks)
                            a_chunk(kt, qg)
                            if qg == 3:
                                a_finish(kt)
            nc.sync.dma_start(invS_d[:], invs_all[:])

    if not nc.is_finalized():
        nc.finalize()
    return nc


_PROGRAM = None


def _get_program():
    global _PROGRAM
    if _PROGRAM is None:
        _PROGRAM = _build_program()
    return _PROGRAM


def make_in_maps(q_input, k_input, v_input, Wq, bq, Wk, bk, Wv, bv):
    q_input = np.asarray(q_input, dtype=np.float32)
    k_input = np.asarray(k_input, dtype=np.float32)
    v_input = np.asarray(v_input, dtype=np.float32)
    wqT = np.asarray(Wq, dtype=np.float32).T * SCALE
    wkT = np.asarray(Wk, dtype=np.float32).T
    wvT = np.asarray(Wv, dtype=np.float32).T
    wAll = np.empty((2, P, 3 * E), dtype=np.float32)
    for dc in range(2):
        rows = slice(dc * P, (dc + 1) * P)
        wAll[dc, :, 0:E] = wqT[rows]
        wAll[dc, :, E:2 * E] = wkT[rows]
        wAll[dc, :, 2 * E:3 * E] = wvT[rows]
    bq16 = np.asarray(bq, dtype=np.float32) * SCALE
    bk_f = np.asarray(bk, dtype=np.float32)
    bqk = np.stack([bq16[0:P], bq16[P:2 * P], bk_f[0:P], bk_f[P:2 * P]],
                   axis=1).astype(np.float32)
    bv_r = np.broadcast_to(np.asarray(bv, dtype=np.float32), (P, E)).copy()

    in_maps = []
    for c in range(NC_CORES):
        b, h = c // 2, c % 2
        kr = slice(h * KL, (h + 1) * KL)
        in_maps.append({
            "qT": np.ascontiguousarray(q_input[b].T),
            "kT_src": np.ascontiguousarray(v_input[b, kr].T),  # K from v_input
            "vT_src": np.ascontiguousarray(k_input[b, kr].T),  # V from k_input
            "wAll": wAll, "bqk": bqk, "bv": bv_r,
        })
    return in_maps


def assemble_outputs(results):
    attn = np.empty((B, S, S), dtype=np.float32)
    out = np.zeros((B, S, E), dtype=np.float32)
    for c in range(NC_CORES):
        b, h = c // 2, c % 2
        kr = slice(h * KL, (h + 1) * KL)
        inv = np.asarray(results[c]["invS"]).T.reshape(KL)[:, None]
        attn[b, :, kr] = (np.asarray(results[c]["attn"]).astype(np.float32)
                          * inv).T
        out[b] += np.asarray(results[c]["outT"]).astype(np.float32).T
    return out, attn


def run_sharded(inputs, trace=False, **spmd_kwargs):
    from concourse.bass_utils import run_bass_kernel_spmd
    nc = _get_program()
    in_maps = make_in_maps(**inputs)
    res = run_bass_kernel_spmd(nc, in_maps, core_ids=list(range(NC_CORES)),
                               trace=trace, **spmd_kwargs)
    out, attn = assemble_outputs(res.results)
    return (out, attn), res


def kernel(**inputs):
    (out, attn), _ = run_sharded(inputs, trace=False)
    return out, attn
